# revision 17
# baseline (speedup 1.0000x reference)
"""Trainium2 Bass kernel for the ContextComputer GNN message-passing module.

Computation (per batch row b):
    W1, W2 = W[:D], W[D:]
    u_j    = memory_j * mask_j                       # [N, D]
    a_i    = memory_i @ W1 + bias                    # [N, D]
    c_j    = mask_j * (memory_j @ W2)                # [N, D]
    ctx_i  = sum_{j != i} sigmoid(a_i + c_j) * u_j

Distribution: the whole batch runs on ONE NeuronCore. In this axon-
tunneled PJRT environment the per-call dispatch overhead scales with the
number of devices in the sharded computation (~1.1 ms per extra device
at steady state) while the device compute for the full B=8192 problem is
only ~1.5-3 ms, so a single-core kernel beats the 8-way data-parallel
split end to end by ~3.5x.

Per-core kernel layout: batch rows on the 128 SBUF partitions (64 row
blocks), feature dim (D=512) on the free axis.
  - memory/W/b are pre-cast to bf16 on the host; the kernel streams
    bf16 and writes a bf16 output that the host upcasts (tolerance is
    rel_err < 2e-2; bf16 compute adds ~4e-3).
  - m^T tiles for the matmuls come from ONE whole-block SBUF->SBUF
    xbar transpose DMA per block ([128, 3072] -> [128, 24, 128]) on the
    sync HWDGE ring - no DRAM bf16 scratch round-trip.
  - u_j = mask_j * m_j on ScalarE (per-partition scale); a'_i = m_i @ W1
    + 1*bias via PSUM accumulation (bias via a rank-1 ones matmul);
    c_j = mask_j * (m_j @ W2) applies the mask as a per-partition scale
    in the ScalarE PSUM->SBUF copy.
  - Pairwise stage: one wide DVE/ACT instruction per two i over all 6 j;
    the off-diagonal 5-term j-sum is a strided paired-tree on the DVE
    (3 tensor_tensor ops per i), the last op writing straight into the
    bf16 out tile; one store DMA per block. Emission is software-
    pipelined with a 1-block skew (front-end of bt before pairwise of
    bt-1) and block transposes alternate between the two HWDGE rings.

Measured steady state (device-resident donate-chain, deep pipeline):
~2.32 ms/call vs the 8-core baseline's 11.20 ms. Variants with the
j-sum on TensorE (identity-lhsT PSUM accumulation), single vs dual
transpose rings, and unskewed emission all measure 2.32-2.43 ms - the
residual is DVE elementwise work plus ~0.6-0.9 ms per-call axon/PJRT
dispatch overhead.
"""

import numpy as np
import ml_dtypes

import concourse.bass as bass
import concourse.mybir as mybir
import concourse.tile as tile
from concourse.bass_utils import run_bass_kernel_spmd

B, N, D = 8192, 6, 512
P = 128
DC = D // P  # 4 contraction chunks of 128
NBT = B // P  # 64 row blocks
NCORES = 1
BLOC = B

F32 = mybir.dt.float32
BF16 = mybir.dt.bfloat16

_ADD = mybir.AluOpType.add
_MULT = mybir.AluOpType.mult
_SIGMOID = mybir.ActivationFunctionType.Sigmoid

# per-i strided pairing of the 5 off-diagonal blocks: s = p[in0] + p[in1]
# (two blocks each, uniform strides), then s0+s1, then + p[rem]
_PAIRS = {
    0: (slice(1, 4, 2), slice(2, 5, 2), 5),
    1: (slice(0, 4, 3), slice(2, 5, 2), 5),
    2: (slice(0, 4, 3), slice(1, 5, 3), 5),
    3: (slice(0, 3, 2), slice(1, 5, 3), 5),
    4: (slice(0, 3, 2), slice(1, 4, 2), 5),
    5: (slice(0, 3, 2), slice(1, 4, 2), 4),
}

_nc_cache = {}


def _split_excess_waits(nc, max_waits=1):
    """The pinned walrus build only supports one sync-wait slot per
    instruction; hoist extra Tile-emitted waits onto standalone
    same-engine EventSemaphore instructions (NX dispatcher-level waits,
    so ordering semantics are preserved)."""
    f = nc.m.functions[0]
    for blk in f.blocks:
        new = []
        for ins in blk.instructions:
            si = getattr(ins, "sync_info", None)
            eng = getattr(ins, "engine", None)
            if si is not None and si.on_wait and len(si.on_wait) > max_waits and eng is not None:
                waits = list(si.on_wait)
                extra, keep = waits[:-max_waits], waits[-max_waits:]
                for k, w in enumerate(extra):
                    new.append(
                        mybir.InstEventSemaphore(
                            name=f"{ins.name}_xw{k}",
                            opcode="EventSemaphore",
                            engine=eng,
                            ins=[],
                            outs=[],
                            sync_info=mybir.SyncInfo(on_wait=[w], on_update=[]),
                        )
                    )
                si.on_wait = keep
            new.append(ins)
        blk.instructions[:] = new


def build(bloc=BLOC, split_waits=True, transpose_rings=1, skip_transpose=False):
    nbt = bloc // P
    nc = bass.Bass(num_swdge_queues=4)
    mem = nc.declare_dram_parameter("memory", [bloc, N, D], BF16, isOutput=False)
    msk = nc.declare_dram_parameter("mask", [bloc, N, 1], F32, isOutput=False)
    w_p = nc.declare_dram_parameter("W", [2 * D, D], BF16, isOutput=False)
    b_p = nc.declare_dram_parameter("b", [D], BF16, isOutput=False)
    out = nc.declare_dram_parameter("context", [bloc, N, D], BF16, isOutput=True)

    with tile.TileContext(nc) as tc:
        with (
            tc.tile_pool(name="const", bufs=1) as constp,
            tc.tile_pool(name="mload", bufs=3) as mlp,
            tc.tile_pool(name="mt", bufs=2) as mtp,
            tc.tile_pool(name="ac", bufs=2) as acp,
            tc.tile_pool(name="pair", bufs=2) as pairp,
            tc.tile_pool(name="up", bufs=2) as upool,
            tc.tile_pool(name="maskp", bufs=4) as maskp,
            tc.tile_pool(name="outp", bufs=2) as outp,
            tc.tile_pool(name="acc", bufs=2) as accp,
            tc.tile_pool(name="psA", bufs=3, space="PSUM") as psA,
            tc.tile_pool(name="psC", bufs=3, space="PSUM") as psC,
        ):
            # ---- constants: W1/W2 tiles, bias, ones row, identity ----
            wt = {}
            for h in range(2):  # 0 -> W1, 1 -> W2
                for dc in range(DC):
                    t = constp.tile([P, D], BF16, tag=f"w{h}{dc}")
                    nc.gpsimd.dma_start(
                        out=t[:], in_=w_p[h * D + dc * P : h * D + (dc + 1) * P, :]
                    )
                    wt[h, dc] = t
            bias_t = constp.tile([1, D], BF16, tag="bias")
            nc.gpsimd.dma_start(out=bias_t[:], in_=b_p[None, :])
            ones_t = constp.tile([1, P], BF16, tag="ones")
            nc.vector.memset(ones_t[:], 1.0)

            # Software-pipelined emission with a 1-block skew: each engine's
            # in-order stream sees block bt's independent front-end work
            # BEFORE block bt-1's pairwise stage, so the PE never stalls at
            # the tree matmuls waiting on the DVE/ACT chain (and vice versa).
            state = {}

            def emit_front(bt):
                bsl = slice(bt * P, (bt + 1) * P)
                # ---- load block (SWDGE), whole-block SBUF->SBUF transpose ----
                m_all = mlp.tile([P, N * D], BF16, tag="m")
                nc.gpsimd.dma_start(
                    out=m_all.rearrange("p (n d) -> p n d", n=N), in_=mem[bsl]
                )
                mask_t = maskp.tile([P, N], F32, tag="mask")
                nc.gpsimd.dma_start(out=mask_t[:], in_=msk[bsl, :, 0])
                # [128 rows, 3072] -> [128 d, 24, 128 rows]; chunk k = (j, dc)
                if skip_transpose:  # timing ablation only - zeros, never rewritten
                    mt_all = constp.tile([P, N * DC, P], BF16, tag="mtz")
                    if bt == 0:
                        nc.vector.memset(mt_all[:], 0.0)
                elif transpose_rings == 2:
                    mt_all = mtp.tile([P, N * DC, P], BF16, tag="mt")
                    half = N * DC // 2
                    nc.sync.dma_start(
                        out=mt_all[:, :half, :],
                        in_=m_all[:, : half * P],
                        transpose=True,
                    )
                    nc.scalar.dma_start(
                        out=mt_all[:, half:, :],
                        in_=m_all[:, half * P :],
                        transpose=True,
                    )
                else:
                    # alternate blocks between the two HWDGE rings so the
                    # xbar transposes of consecutive blocks run concurrently
                    mt_all = mtp.tile([P, N * DC, P], BF16, tag="mt")
                    ring = nc.sync if bt % 2 == 0 else nc.scalar
                    ring.dma_start(out=mt_all[:], in_=m_all[:], transpose=True)

                # u_j = mask_j * m_j (ScalarE per-partition scale)
                u_all = upool.tile([P, N * D], BF16, tag="u")
                for j in range(N):
                    nc.scalar.mul(
                        out=u_all[:, j * D : (j + 1) * D],
                        in_=m_all[:, j * D : (j + 1) * D],
                        mul=mask_t[:, j : j + 1],
                    )

                # ---- matmuls ----
                a_all = acp.tile([P, N * D], BF16, tag="a")
                c_all = acp.tile([P, N * D], BF16, tag="c")
                for i in range(N):
                    a_ps = psA.tile([P, D], F32, tag="aps")
                    for dc in range(DC):
                        nc.tensor.matmul(
                            out=a_ps[:],
                            lhsT=mt_all[:, i * DC + dc, :],
                            rhs=wt[0, dc][:],
                            start=(dc == 0),
                            stop=False,
                        )
                    nc.tensor.matmul(
                        out=a_ps[:],
                        lhsT=ones_t[:],
                        rhs=bias_t[:],
                        start=False,
                        stop=True,
                    )
                    nc.scalar.copy(out=a_all[:, i * D : (i + 1) * D], in_=a_ps[:])
                for j in range(N):
                    c_ps = psC.tile([P, D], F32, tag="cps")
                    for dc in range(DC):
                        nc.tensor.matmul(
                            out=c_ps[:],
                            lhsT=mt_all[:, j * DC + dc, :],
                            rhs=wt[1, dc][:],
                            start=(dc == 0),
                            stop=(dc == DC - 1),
                        )
                    # c_j = mask_j * (m_j @ W2): scale in the PSUM->SBUF copy
                    nc.scalar.mul(
                        out=c_all[:, j * D : (j + 1) * D],
                        in_=c_ps[:],
                        mul=mask_t[:, j : j + 1],
                    )
                state[bt] = (u_all, a_all, c_all)

            def emit_pairwise(bt):
                bsl = slice(bt * P, (bt + 1) * P)
                u_all, a_all, c_all = state.pop(bt)
                # ---- pairwise sigmoid gating, two i per instruction ----
                out_all = outp.tile([P, N * D], BF16, tag="o")
                for i0 in range(0, N, 2):
                    # t[(i,j)] = a_i + c_j for i in {i0, i0+1}, all j
                    a_b = (
                        a_all[:, i0 * D : (i0 + 2) * D]
                        .rearrange("p (i f) -> p i f", i=2)
                        .rearrange("p i (j f) -> p i j f", j=1)
                        .broadcast_to([P, 2, N, D])
                    )
                    c_b = (
                        c_all.rearrange("p (i f) -> p i f", i=1)
                        .broadcast_to([P, 2, N * D])
                        .rearrange("p i (j f) -> p i j f", j=N)
                    )
                    t_all = pairp.tile([P, 2 * N * D], BF16, tag="t")
                    nc.vector.tensor_tensor(
                        out=t_all.rearrange("p (i j f) -> p i j f", i=2, j=N),
                        in0=a_b,
                        in1=c_b,
                        op=_ADD,
                    )
                    g_all = pairp.tile([P, 2 * N * D], BF16, tag="g")
                    nc.scalar.activation(out=g_all[:], in_=t_all[:], func=_SIGMOID)
                    u_b = (
                        u_all.rearrange("p (i f) -> p i f", i=1)
                        .broadcast_to([P, 2, N * D])
                    )
                    p_all = pairp.tile([P, 2 * N * D], BF16, tag="pp")
                    nc.vector.tensor_tensor(
                        out=p_all.rearrange("p (i f) -> p i f", i=2),
                        in0=g_all.rearrange("p (i f) -> p i f", i=2),
                        in1=u_b,
                        op=_MULT,
                    )
                    # off-diagonal 5-term j-sum: strided paired-tree on DVE
                    # (3 TT ops per i), last op writing into the out tile
                    for il in range(2):
                        i = i0 + il
                        pv = p_all[:, il * N * D : (il + 1) * N * D].rearrange(
                            "p (j f) -> p j f", j=N
                        )
                        # on GpSimd: keeps the adder trees off the DVE, whose
                        # two wide TT passes are the critical path
                        s0, s1, rem = _PAIRS[i]
                        s = accp.tile([P, 2 * D], BF16, tag="s")
                        nc.gpsimd.tensor_tensor(
                            out=s.rearrange("p (j f) -> p j f", j=2),
                            in0=pv[:, s0, :],
                            in1=pv[:, s1, :],
                            op=_ADD,
                        )
                        s2 = accp.tile([P, D], BF16, tag="s2")
                        nc.gpsimd.tensor_add(out=s2[:], in0=s[:, :D], in1=s[:, D:])
                        nc.gpsimd.tensor_add(
                            out=out_all[:, i * D : (i + 1) * D],
                            in0=s2[:],
                            in1=pv[:, rem, :],
                        )
                nc.gpsimd.dma_start(
                    out=out[bsl], in_=out_all.rearrange("p (n d) -> p n d", n=N)
                )

            for bt in range(nbt + 1):
                if bt < nbt:
                    emit_front(bt)
                if bt >= 1:
                    emit_pairwise(bt - 1)
    if split_waits:
        _split_excess_waits(nc)
    return nc


def get_nc(bloc=BLOC):
    if bloc not in _nc_cache:
        _nc_cache[bloc] = build(bloc)
    return _nc_cache[bloc]


def make_in_maps(inputs):
    """Host-side input staging: pre-cast to the kernel's storage dtypes."""
    memory = np.asarray(inputs["memory"], dtype=np.float32)
    mask = np.ascontiguousarray(np.asarray(inputs["mask"], dtype=np.float32))
    W = np.asarray(inputs["W"], dtype=np.float32)
    b = np.asarray(inputs["b"], dtype=np.float32)
    return [
        {
            "memory": np.ascontiguousarray(memory).astype(ml_dtypes.bfloat16),
            "mask": mask,
            "W": np.ascontiguousarray(W).astype(ml_dtypes.bfloat16),
            "b": np.ascontiguousarray(b).astype(ml_dtypes.bfloat16),
        }
    ]


last_results = None


def kernel(**inputs):
    global last_results
    nc = get_nc()
    in_maps = make_in_maps(inputs)
    res = run_bass_kernel_spmd(nc, in_maps, list(range(NCORES)))
    last_results = res
    out = res.results[0]["context"]
    return np.asarray(out).astype(np.float32)
